# revision 1
# baseline (speedup 1.0000x reference)
"""Trainium2 Bass kernel for nn_AALSTM (2-layer closed-loop LSTM, B=128 T=1024).

Strategy: data-parallel over batch across 8 NeuronCores (16 samples/core, no
per-step communication). Per core, the T=1024 sequential scan is fully
unrolled. Weights are the stationary matmul operand in bf16 (fast weight
load); moving operands are [128,16] state slices. Gates accumulate in PSUM as
[128 partitions, 16*chunk] tiles so activations/elementwise run across all
128 lanes. The closed-loop feedback (fc output -> next-step input feature 31)
is a single [1,16] copy into row 31 of the transposed input buffer. BatchNorm
stats are computed on device with one small AllReduce. The final FC over all
timesteps is batched at the end from an SBUF-resident h1 history.
"""

import os
import sys

sys.path.insert(0, "/opt/trn_rl_repo")

import numpy as np
import ml_dtypes

import concourse.bass as bass
import concourse.mybir as mybir
from concourse.bass_utils import run_bass_kernel_spmd

B, T_FULL, F = 128, 1024, 32
H1, H2 = 512, 256
G1, G2 = 4 * H1, 4 * H2  # 2048, 1024
EPS = 1e-5
N_CORES = 8
BL = B // N_CORES  # 16 local batch

T = int(os.environ.get("AALSTM_T", T_FULL))

F32 = mybir.dt.float32
BF16 = mybir.dt.bfloat16
AX = mybir.AxisListType
ALU = mybir.AluOpType
AF = mybir.ActivationFunctionType

_NC_CACHE = {}


def build_nc(t_steps: int) -> bass.Bass:
    nc = bass.Bass(target_bir_lowering=False, num_devices=N_CORES)
    NBT = t_steps * BL  # columns of the transposed input

    # ---- DRAM parameters ----
    xT_d = nc.declare_dram_parameter("xT", [F, NBT], F32, isOutput=False)
    wih1_d = nc.declare_dram_parameter("wih1", [128, G1], BF16, isOutput=False)
    whh1_d = nc.declare_dram_parameter("whh1", [128, 4 * G1], BF16, isOutput=False)
    wih2_d = nc.declare_dram_parameter("wih2", [128, 4 * G2], BF16, isOutput=False)
    whh2_d = nc.declare_dram_parameter("whh2", [128, 2 * G2], BF16, isOutput=False)
    wfc_d = nc.declare_dram_parameter("wfc", [128, 64], BF16, isOutput=False)
    ident_d = nc.declare_dram_parameter("ident", [128, 128], BF16, isOutput=False)
    b2bc_d = nc.declare_dram_parameter("b2bc", [128, 128], BF16, isOutput=False)
    b1bc_d = nc.declare_dram_parameter("b1bc", [128, 256], BF16, isOutput=False)
    consts_d = nc.declare_dram_parameter("consts", [128, 8], F32, isOutput=False)
    out_d = nc.declare_dram_parameter("out", [t_steps, BL], F32, isOutput=True)

    # internal DRAM bounce buffers for the BN all-reduce
    bnin_d = nc.dram_tensor("bnin", [F, 2], F32)
    bnout_d = nc.dram_tensor("bnout", [F, 2], F32)

    DEBUG = os.environ.get("AALSTM_DEBUG") == "1"
    if DEBUG:
        dbg_bnst_d = nc.declare_dram_parameter("dbg_bnst", [F, 8], F32, isOutput=True)
        dbg_bng_d = nc.declare_dram_parameter("dbg_bng", [F, 2], F32, isOutput=True)
        dbg_xnb_d = nc.declare_dram_parameter("dbg_xnb", [128, 16], BF16, isOutput=True)
        dbg_sact1_d = nc.declare_dram_parameter("dbg_sact1", [128, 256], F32, isOutput=True)
        dbg_sact2_d = nc.declare_dram_parameter("dbg_sact2", [128, 128], F32, isOutput=True)
        dbg_h0b_d = nc.declare_dram_parameter("dbg_h0b", [128, 64], BF16, isOutput=True)
        dbg_hist_d = nc.declare_dram_parameter("dbg_hist", [128, 32], BF16, isOutput=True)

    import contextlib

    with contextlib.ExitStack() as st:
        ctx = st.enter_context

        # ---- SBUF ----
        xT = ctx(nc.sbuf_tensor("s_xT", [F, NBT], F32))
        xnb = ctx(nc.sbuf_tensor("xnb", [128, NBT + 32], BF16))
        w_ih1 = ctx(nc.sbuf_tensor("w_ih1", [128, G1], BF16))
        w_hh1 = ctx(nc.sbuf_tensor("w_hh1", [128, 4 * G1], BF16))
        w_ih2 = ctx(nc.sbuf_tensor("w_ih2", [128, 4 * G2], BF16))
        w_hh2 = ctx(nc.sbuf_tensor("w_hh2", [128, 2 * G2], BF16))
        w_fc = ctx(nc.sbuf_tensor("w_fc", [128, 64], BF16))
        identb = ctx(nc.sbuf_tensor("identb", [128, 128], BF16))
        b2bc = ctx(nc.sbuf_tensor("s_b2bc", [128, 128], BF16))
        b1bc = ctx(nc.sbuf_tensor("s_b1bc", [128, 256], BF16))
        consts = ctx(nc.sbuf_tensor("s_consts", [128, 8], F32))
        hist = ctx(nc.sbuf_tensor("hist", [128, 64], BF16))  # 2 parities x 32
        oring = ctx(nc.sbuf_tensor("oring", [32, 64], F32))  # 4-slot out ring
        h0b = ctx(nc.sbuf_tensor("h0b", [128, 128], BF16))  # 2 parities x 64
        c0 = ctx(nc.sbuf_tensor("c0", [128, 64], F32))
        c1 = ctx(nc.sbuf_tensor("c1", [128, 32], F32))
        sact1 = ctx(nc.sbuf_tensor("sact1", [128, 512], F32))  # 2 par x 256
        sact2 = ctx(nc.sbuf_tensor("sact2", [128, 256], F32))  # 2 par x 128
        tnh0 = ctx(nc.sbuf_tensor("tnh0", [128, 128], F32))  # 2 par x 64
        tnh1 = ctx(nc.sbuf_tensor("tnh1", [128, 64], F32))  # 2 par x 32
        tmp = ctx(nc.sbuf_tensor("tmp", [128, 128], F32))
        bnin = ctx(nc.sbuf_tensor("s_bnin", [F, 2], F32))
        bng = ctx(nc.sbuf_tensor("bng", [F, 2], F32))
        bnst = ctx(nc.sbuf_tensor("bnst", [F, 8], F32))

        # ---- PSUM (6 banks) ----
        g1p = [ctx(nc.psum_tensor(f"g1p{i}", [128, 512], F32)) for i in range(2)]
        g2p = [ctx(nc.psum_tensor(f"g2p{i}", [128, 512], F32)) for i in range(2)]
        fcp = [ctx(nc.psum_tensor(f"fcp{i}", [128, 512], F32)) for i in range(2)]

        # ---- semaphores ----
        dsem_x = ctx(nc.semaphore("dsem_x"))
        dsem_w = ctx(nc.semaphore("dsem_w"))
        dsem_bni = ctx(nc.semaphore("dsem_bni"))
        dsem_bn = ctx(nc.semaphore("dsem_bn"))
        dsem_out = ctx(nc.semaphore("dsem_out"))
        s_bn = ctx(nc.semaphore("s_bn"))
        inits = ctx(nc.semaphore("inits"))
        sPE = ctx(nc.semaphore("sPE"))
        sACT = ctx(nc.semaphore("sACT"))
        sDVE = ctx(nc.semaphore("sDVE"))

        block = ctx(nc.Block())

        # ---------------- DMA / sync engine ----------------
        @block.sync
        def _(sync):
            sync.dma_start(xT[:, :], xT_d[:, :]).then_inc(dsem_x, 16)
            sync.dma_start(w_ih1[:, :], wih1_d[:, :]).then_inc(dsem_w, 16)
            sync.dma_start(w_hh1[:, :], whh1_d[:, :]).then_inc(dsem_w, 16)
            sync.dma_start(w_ih2[:, :], wih2_d[:, :]).then_inc(dsem_w, 16)
            sync.dma_start(w_hh2[:, :], whh2_d[:, :]).then_inc(dsem_w, 16)
            sync.dma_start(w_fc[:, :], wfc_d[:, :]).then_inc(dsem_w, 16)
            sync.dma_start(identb[:, :], ident_d[:, :]).then_inc(dsem_w, 16)
            sync.dma_start(b2bc[:, :], b2bc_d[:, :]).then_inc(dsem_w, 16)
            sync.dma_start(b1bc[:, :], b1bc_d[:, :]).then_inc(dsem_w, 16)
            sync.dma_start(consts[:, :], consts_d[:, :]).then_inc(dsem_w, 16)
            # BN bounce out / in
            sync.wait_ge(s_bn, 2)
            sync.dma_start(bnin_d[:, :], bnin[:, :]).then_inc(dsem_bni, 16)
            sync.wait_ge(s_bn, 3)
            sync.dma_start(bng[:, :], bnout_d[:, :]).then_inc(dsem_bn, 16)
            # per-step output rows (t-major), 4-deep ring
            for t in range(t_steps):
                sync.wait_ge(sACT, 10 * t + 10)
                sync.dma_start(
                    out_d[t : t + 1, :], oring[0:1, 16 * (t % 4) : 16 * (t % 4) + 16]
                ).then_inc(dsem_out, 16)
            if DEBUG:
                sync.wait_ge(sACT, 10 * t_steps)
                sync.wait_ge(sDVE, 8 * t_steps)
                sync.dma_start(dbg_bnst_d[:, :], bnst[:, :]).then_inc(dsem_out, 16)
                sync.dma_start(dbg_bng_d[:, :], bng[:, :]).then_inc(dsem_out, 16)
                sync.dma_start(dbg_xnb_d[:, :], xnb[:, 0:16]).then_inc(dsem_out, 16)
                sync.dma_start(dbg_sact1_d[:, :], sact1[:, 0:256]).then_inc(dsem_out, 16)
                sync.dma_start(dbg_sact2_d[:, :], sact2[:, 0:128]).then_inc(dsem_out, 16)
                sync.dma_start(dbg_h0b_d[:, :], h0b[:, 0:64]).then_inc(dsem_out, 16)
                sync.dma_start(dbg_hist_d[:, :], hist[:, 0:32]).then_inc(dsem_out, 16)
                sync.wait_ge(dsem_out, 16 * t_steps + 112)
            else:
                sync.wait_ge(dsem_out, 16 * t_steps)

        # ---------------- GPSIMD: memsets + collective ----------------
        @block.gpsimd
        def _(gpsimd):
            gpsimd.memset(xnb[32:64, :], 0.0).then_inc(inits, 1)
            gpsimd.memset(xnb[64:128, :], 0.0).then_inc(inits, 1)
            gpsimd.memset(xnb[32:33, :], 1.0).then_inc(inits, 1)
            gpsimd.memset(c0[:, :], 0.0).then_inc(inits, 1)
            gpsimd.memset(c1[:, :], 0.0).then_inc(inits, 1)
            gpsimd.wait_ge(dsem_bni, 16)
            gpsimd.collective_compute(
                "AllReduce",
                ALU.add,
                replica_groups=[list(range(N_CORES))],
                ins=[bnin_d.ap()],
                outs=[bnout_d.ap()],
            ).then_inc(s_bn, 1)

        # ---------------- VectorE ----------------
        @block.vector
        def _(vector):
            # BN partial sums
            vector.wait_ge(dsem_x, 16)
            vector.tensor_reduce(
                bnin[:, 0:1], xT[:, :], axis=AX.X, op=ALU.add
            ).then_inc(s_bn, 1)
            # stats finalize (after all-reduce round trip)
            vector.wait_ge(dsem_bn, 16)
            inv = 1.0 / float(B * T_FULL) if t_steps == T_FULL else 1.0 / float(B * t_steps)
            vector.tensor_scalar_mul(bnst[:, 0:1], bng[:, 0:1], inv)  # mean
            vector.tensor_scalar_mul(bnst[:, 1:2], bng[:, 1:2], inv)  # E[x^2]
            vector.tensor_scalar(
                bnst[:, 2:3], bnst[:, 0:1], bnst[:, 0:1], None, op0=ALU.mult
            )  # mean^2  (scalar operand written 2 ops back: OK)
            # NOTE: TensorScalarPtr scalar operands are prefetched early; a
            # scalar written by the immediately preceding DVE op reads stale
            # data. Space such chains with dummy ops.
            vector.tensor_scalar_mul(bnst[:, 7:8], bnst[:, 0:1], 1.0)  # spacer
            vector.tensor_scalar_mul(bnst[:, 7:8], bnst[:, 0:1], 1.0)  # spacer
            vector.tensor_scalar(
                bnst[:, 2:3], bnst[:, 1:2], bnst[:, 2:3], EPS,
                op0=ALU.subtract, op1=ALU.add
            ).then_inc(s_bn, 1)  # var + eps
            # ACT computes sqrt -> bnst[:,3]
            vector.wait_ge(s_bn, 5)
            vector.reciprocal(bnst[:, 4:5], bnst[:, 3:4])
            vector.tensor_scalar_mul(bnst[:, 7:8], bnst[:, 0:1], 1.0)  # spacer
            vector.tensor_scalar_mul(bnst[:, 7:8], bnst[:, 0:1], 1.0)  # spacer
            vector.tensor_scalar(
                bnst[:, 5:6], consts[0:F, 0:1], bnst[:, 4:5], None, op0=ALU.mult
            )  # a
            vector.tensor_scalar_mul(bnst[:, 7:8], bnst[:, 0:1], 1.0)  # spacer
            vector.tensor_scalar_mul(bnst[:, 7:8], bnst[:, 0:1], 1.0)  # spacer
            vector.tensor_scalar(
                bnst[:, 6:7], bnst[:, 0:1], bnst[:, 5:6], -1.0,
                op0=ALU.mult, op1=ALU.mult
            )  # -mean*a
            vector.tensor_scalar(
                bnst[:, 6:7], bnst[:, 6:7], consts[0:F, 1:2], None, op0=ALU.add
            ).then_inc(s_bn, 1)  # b = beta - mean*a

            # ---- scan ----
            for t in range(t_steps):
                p = t % 2
                a1 = 256 * p
                a2 = 128 * p
                vector.wait_ge(sACT, 10 * t + 2)
                vector.tensor_mul(tmp[:, 0:64], sact1[:, a1 + 0 : a1 + 64],
                    sact1[:, a1 + 128 : a1 + 192]).then_inc(sDVE, 1)
                vector.tensor_mul(tmp[:, 64:128], sact1[:, a1 + 64 : a1 + 128],
                    c0[:, :]).then_inc(sDVE, 1)
                vector.tensor_add(c0[:, :], tmp[:, 0:64], tmp[:, 64:128]).then_inc(sDVE, 1)
                vector.wait_ge(sACT, 10 * t + 4)
                vector.tensor_mul(h0b[:, 64 * p : 64 * p + 64], sact1[:, a1 + 192 : a1 + 256],
                    tnh0[:, 64 * p : 64 * p + 64]).then_inc(sDVE, 1)
                vector.wait_ge(sACT, 10 * t + 6)
                vector.tensor_mul(tmp[:, 0:32], sact2[:, a2 + 0 : a2 + 32],
                    sact2[:, a2 + 64 : a2 + 96]).then_inc(sDVE, 1)
                vector.tensor_mul(tmp[:, 32:64], sact2[:, a2 + 32 : a2 + 64],
                    c1[:, :]).then_inc(sDVE, 1)
                vector.tensor_add(c1[:, :], tmp[:, 0:32], tmp[:, 32:64]).then_inc(sDVE, 1)
                vector.wait_ge(sACT, 10 * t + 8)
                vector.tensor_mul(hist[:, 32 * p : 32 * p + 32], sact2[:, a2 + 96 : a2 + 128],
                    tnh1[:, 32 * p : 32 * p + 32]).then_inc(sDVE, 1)

        # ---------------- ScalarE ----------------
        @block.scalar
        def _(scalar):
            act = scalar.activation
            # BN: sum of squares via Square-activation accumulate
            scalar.wait_ge(dsem_x, 16)
            act(xnb[0:F, 0:NBT], xT[:, :], AF.Square,
                accum_out=bnin[:, 1:2]).then_inc(s_bn, 1)
            scalar.wait_ge(s_bn, 4)
            act(bnst[:, 3:4], bnst[:, 2:3], AF.Sqrt).then_inc(s_bn, 1)
            scalar.wait_ge(s_bn, 6)
            scalar.wait_ge(inits, 5)
            act(xnb[0:F, 0:NBT], xT[:, :], AF.Identity,
                bias=bnst[:, 6:7], scale=bnst[:, 5:6]).then_inc(inits, 1)

            for t in range(t_steps):
                p = t % 2
                a1 = 256 * p
                a2 = 128 * p
                scalar.wait_ge(sPE, 3 * t + 1)
                act(sact1[:, a1 + 0 : a1 + 128], g1p[p][:, 0:128], AF.Sigmoid).then_inc(sACT, 1)
                act(sact1[:, a1 + 128 : a1 + 192], g1p[p][:, 128:192], AF.Tanh).then_inc(sACT, 1)
                act(sact1[:, a1 + 192 : a1 + 256], g1p[p][:, 192:256], AF.Sigmoid).then_inc(sACT, 1)
                scalar.wait_ge(sDVE, 8 * t + 3)
                act(tnh0[:, 64 * p : 64 * p + 64], c0[:, :], AF.Tanh).then_inc(sACT, 1)
                scalar.wait_ge(sPE, 3 * t + 2)
                act(sact2[:, a2 + 0 : a2 + 64], g2p[p][:, 0:64], AF.Sigmoid).then_inc(sACT, 1)
                act(sact2[:, a2 + 64 : a2 + 96], g2p[p][:, 64:96], AF.Tanh).then_inc(sACT, 1)
                act(sact2[:, a2 + 96 : a2 + 128], g2p[p][:, 96:128], AF.Sigmoid).then_inc(sACT, 1)
                scalar.wait_ge(sDVE, 8 * t + 7)
                act(tnh1[:, 32 * p : 32 * p + 32], c1[:, :], AF.Tanh).then_inc(sACT, 1)
                scalar.wait_ge(sPE, 3 * t + 3)
                act(xnb[0:1, BL * (t + 1) : BL * (t + 2)], fcp[p][0:1, 0:BL],
                    AF.Identity, bias=consts[0:1, 2:3]).then_inc(sACT, 1)
                if t >= 4:
                    scalar.wait_ge(dsem_out, 16 * (t - 3))
                act(oring[0:1, 16 * (t % 4) : 16 * (t % 4) + 16], fcp[p][0:1, 0:BL],
                    AF.Identity, bias=consts[0:1, 2:3]).then_inc(sACT, 1)

        # ---------------- TensorE ----------------
        @block.tensor
        def _(tensor):
            def mm(out, lhsT, rhs, start, stop=False):
                return tensor.matmul(out, lhsT, rhs, start=start, stop=stop,
                                     skip_group_check=True)

            def emit_fc(ft):
                # fc of step ft: by emission point the h1' chain has run
                # under the cover of the next step's Whh1 group
                fp = ft % 2
                tensor.wait_ge(sDVE, 8 * ft + 8)
                mm(fcp[fp][0:1, 0:BL], w_fc[:, 0:1],
                   hist[:, 32 * fp : 32 * fp + 16], start=True)
                mm(fcp[fp][0:1, 0:BL], w_fc[:, 1:2],
                   hist[:, 32 * fp + 16 : 32 * fp + 32],
                   start=False, stop=True).then_inc(sPE, 1)

            tensor.wait_ge(inits, 6)
            tensor.wait_ge(dsem_w, 144)
            for t in range(t_steps):
                p = t % 2
                q = (t - 1) % 2
                if t >= 2:
                    tensor.wait_ge(sACT, 10 * (t - 2) + 7)
                mm(g1p[p][:, 0:256], identb[:, :], b1bc[:, :], start=True)
                mm(g2p[p][:, 0:128], identb[:, :], b2bc[:, :], start=True)
                if t >= 1:
                    tensor.wait_ge(sDVE, 8 * (t - 1) + 4)
                    for m in range(16):
                        for kc in range(4):
                            mm(g1p[p][:, 16 * m : 16 * m + 16],
                               w_hh1[:, G1 * kc + 128 * m : G1 * kc + 128 * m + 128],
                               h0b[:, 64 * q + 16 * kc : 64 * q + 16 * kc + 16],
                               start=False)
                    tensor.wait_ge(sDVE, 8 * (t - 1) + 8)
                    for m in range(8):
                        for kc in range(2):
                            mm(g2p[p][:, 16 * m : 16 * m + 16],
                               w_hh2[:, G2 * kc + 128 * m : G2 * kc + 128 * m + 128],
                               hist[:, 32 * q + 16 * kc : 32 * q + 16 * kc + 16],
                               start=False)
                    emit_fc(t - 1)
                # input + feedback + bias contribution to cell1
                if t >= 1:
                    tensor.wait_ge(sACT, 10 * (t - 1) + 9)
                for m in range(16):
                    ins = mm(g1p[p][:, 16 * m : 16 * m + 16],
                             w_ih1[:, 128 * m : 128 * m + 128],
                             xnb[:, BL * t : BL * t + BL],
                             start=False, stop=(m == 15))
                ins.then_inc(sPE, 1)
                # cell2 input part
                tensor.wait_ge(sDVE, 8 * t + 4)
                for m in range(8):
                    for kc in range(4):
                        ins = mm(g2p[p][:, 16 * m : 16 * m + 16],
                                 w_ih2[:, G2 * kc + 128 * m : G2 * kc + 128 * m + 128],
                                 h0b[:, 64 * p + 16 * kc : 64 * p + 16 * kc + 16],
                                 start=False, stop=(m == 7 and kc == 3))
                ins.then_inc(sPE, 1)
            emit_fc(t_steps - 1)

    return nc


def _host_pack(inputs: dict) -> tuple:
    bf = ml_dtypes.bfloat16
    f32 = np.float32
    W_ih1 = np.asarray(inputs["W_ih1"], f32)
    W_hh1 = np.asarray(inputs["W_hh1"], f32)
    W_ih2 = np.asarray(inputs["W_ih2"], f32)
    W_hh2 = np.asarray(inputs["W_hh2"], f32)
    W_fc = np.asarray(inputs["W_fc"], f32)
    b_fc = np.asarray(inputs["b_fc"], f32)
    b1 = np.asarray(inputs["b_ih1"], f32) + np.asarray(inputs["b_hh1"], f32)
    b2 = np.asarray(inputs["b_ih2"], f32) + np.asarray(inputs["b_hh2"], f32)
    gamma = np.asarray(inputs["gamma"], f32)
    beta = np.asarray(inputs["beta"], f32)

    wih1 = np.zeros((128, G1), f32)
    wih1[0] = W_ih1[:, 31]
    wih1[1:F] = W_ih1.T[0:31]
    whh1 = W_hh1.T.reshape(4, 128, G1).transpose(1, 0, 2).reshape(128, 4 * G1)
    wih2 = W_ih2.T.reshape(4, 128, G2).transpose(1, 0, 2).reshape(128, 4 * G2)
    whh2 = W_hh2.T.reshape(2, 128, G2).transpose(1, 0, 2).reshape(128, 2 * G2)
    wfc = np.zeros((128, 64), f32)
    wfc[:, 0] = W_fc[0, 0:128]
    wfc[:, 1] = W_fc[0, 128:256]
    b2bc = np.repeat(b2.reshape(8, 128).T[:, :, None], BL, axis=2).reshape(128, 128)
    b1bc = np.repeat(b1.reshape(16, 128).T[:, :, None], BL, axis=2).reshape(128, 256)
    ident = np.eye(128, dtype=f32)
    consts = np.zeros((128, 8), f32)
    reorder = np.concatenate([[31], np.arange(31)])
    consts[0:F, 0] = gamma[reorder]
    consts[0:F, 1] = beta[reorder]
    consts[:, 2] = b_fc[0]

    shared = {
        "wih1": wih1.astype(bf),
        "whh1": whh1.astype(bf),
        "wih2": wih2.astype(bf),
        "whh2": whh2.astype(bf),
        "wfc": wfc.astype(bf),
        "ident": ident.astype(bf),
        "b2bc": b2bc.astype(bf),
        "b1bc": b1bc.astype(bf),
        "consts": consts,
    }
    return shared


def _prepare(inputs):
    t_steps = T
    x = np.asarray(inputs["x"], np.float32)[:, :t_steps, :]
    shared = _host_pack(inputs)
    if t_steps not in _NC_CACHE:
        _NC_CACHE[t_steps] = build_nc(t_steps)
    nc = _NC_CACHE[t_steps]
    in_maps = []
    for c in range(N_CORES):
        xs = x[c * BL : (c + 1) * BL]  # [BL, T, F]
        xT = xs.transpose(2, 1, 0).reshape(F, t_steps * BL)
        xT = np.ascontiguousarray(np.concatenate([xT[31:32], xT[0:31]], axis=0))
        m = dict(shared)
        m["xT"] = xT
        in_maps.append(m)
    return nc, in_maps


def kernel(**inputs) -> np.ndarray:
    nc, in_maps = _prepare(inputs)
    res = run_bass_kernel_spmd(nc, in_maps, core_ids=list(range(N_CORES)))
    out = np.concatenate([res.results[c]["out"].T for c in range(N_CORES)], axis=0)
    return out.astype(np.float32)


def _install_ntff_hook():
    """The container's antenv lacks axon_hooks; recreate the NTFF profile
    hook (ctypes into libaxon_pjrt.so) so trace=True yields exec_time_ns."""
    import types, ctypes, contextlib as _cl

    try:
        from antenv.axon_hooks import get_axon_ntff_profile_hook  # noqa
        return  # already present
    except ImportError:
        pass
    so_path = "/opt/axon/libaxon_pjrt.so"
    if not os.path.exists(so_path):
        return
    lib = ctypes.CDLL(so_path)
    if not hasattr(lib, "axon_start_nrt_profile"):
        return
    lib.axon_start_nrt_profile.argtypes = [ctypes.POINTER(ctypes.c_int64), ctypes.c_size_t]
    lib.axon_start_nrt_profile.restype = ctypes.c_int64
    lib.axon_stop_nrt_profile.argtypes = [ctypes.c_char_p]
    lib.axon_stop_nrt_profile.restype = ctypes.c_int64

    @_cl.contextmanager
    def _hook(output_dir, device_ids):
        import jax
        jax.devices()
        if device_ids:
            ids = (ctypes.c_int64 * len(device_ids))(*device_ids)
            rc = lib.axon_start_nrt_profile(ids, len(device_ids))
        else:
            rc = lib.axon_start_nrt_profile(None, 0)
        if rc != 0:
            raise RuntimeError(f"axon_start_nrt_profile rc={rc}")
        try:
            yield
        finally:
            n = lib.axon_stop_nrt_profile(str(output_dir).encode())
            print(f"profile: {n} file(s) written to {output_dir}")

    mod = types.ModuleType("antenv.axon_hooks")
    _h = {"hook": _hook}
    mod.set_axon_ntff_profile_hook = lambda h: _h.__setitem__("hook", h)
    mod.get_axon_ntff_profile_hook = lambda: _h["hook"]
    sys.modules["antenv.axon_hooks"] = mod


def timed_run(inputs):
    """Run with tracing enabled; returns neuron-profile exec_time_ns."""
    _install_ntff_hook()
    nc, in_maps = _prepare(inputs)
    res = run_bass_kernel_spmd(nc, in_maps, core_ids=list(range(N_CORES)), trace=True)
    if res.exec_time_ns is None and res.mean_exec_time_ns is not None:
        return int(res.mean_exec_time_ns)
    return res.exec_time_ns


if __name__ == "__main__":
    # simple self-exercise with random inputs
    rng = np.random.default_rng(0)
    demo = dict(
        x=rng.standard_normal((B, T_FULL, F)).astype(np.float32),
        dagger_gt=rng.standard_normal((B, T_FULL)).astype(np.float32),
        W_ih1=rng.standard_normal((G1, F)).astype(np.float32) * 0.04,
        W_hh1=rng.standard_normal((G1, H1)).astype(np.float32) * 0.04,
        b_ih1=rng.standard_normal(G1).astype(np.float32) * 0.04,
        b_hh1=rng.standard_normal(G1).astype(np.float32) * 0.04,
        W_ih2=rng.standard_normal((G2, H1)).astype(np.float32) * 0.06,
        W_hh2=rng.standard_normal((G2, H2)).astype(np.float32) * 0.06,
        b_ih2=rng.standard_normal(G2).astype(np.float32) * 0.06,
        b_hh2=rng.standard_normal(G2).astype(np.float32) * 0.06,
        W_fc=rng.standard_normal((1, H2)).astype(np.float32) * 0.06,
        b_fc=rng.standard_normal(1).astype(np.float32) * 0.06,
        gamma=np.ones(F, np.float32),
        beta=np.zeros(F, np.float32),
    )
    y = kernel(**demo)
    print("kernel output", y.shape, y.dtype, float(np.abs(y).mean()))

